# revision 8
# baseline (speedup 1.0000x reference)
"""Trainium2 Bass kernel for nn_CounterFlowNetwork.

Data-parallel over 8 NeuronCores (batch sharded). Per-core layout:
2048 rows split into 2 chunks of R=1024, activations transposed
([feature, row]) and kept in bf16.

Key restructurings vs the straightforward implementation:
 - Consecutive plate linears folded host-side (f64):
     descending plate = ONE 256x256 matmul (W_desc = alpha*W_tr@W_ab@W_eq)
     ascending  plate = ONE 256x256 matmul (A = -alpha*W_tr)
 - ALL per-plate biases eliminated by a change of variables
   (gamma-fold): stored gas states carry a precomputed constant offset
   gamma_n; every correction lands in a sigmoid pre-activation bias or
   the head bias, which are free (ACT applies bias with the activation).
 - Descending pacc and ascending gas state accumulate directly in PSUM
   across plates (matmul start/stop groups span the whole sweep), so no
   vector adds are needed; sigmoid reads PSUM directly.
 - l[1] for the output head recovered from S = sum of descending driving
   forces (head uses W_fold = alpha*W_tr@W_ab@W1_l).
 - x is cast to bf16 host-side and transposed by the DMA XBAR
   (dma_start_transpose) - no PE transposes at all.
 - The two chunks are emitted plate-interleaved so all engines stay fed
   (in-order sequencers), with PSUM split 4 banks per chunk.
"""

import numpy as np
import ml_dtypes

import concourse.bass as bass
import concourse.bacc as bacc
import concourse.mybir as mybir
import concourse.tile as tile
from concourse import bass_utils

B, D_IN, D_GAS, D_OUT = 16384, 512, 256, 1000
N_PLATES = 8
N_CORES = 8
ROWS = B // N_CORES          # rows per core (2048)
N_CHUNKS = 2
R = ROWS // N_CHUNKS         # rows per chunk (1024)
F32 = mybir.dt.float32
BF16 = mybir.dt.bfloat16
AF = mybir.ActivationFunctionType
OP = mybir.AluOpType
NPBF16 = ml_dtypes.bfloat16


def _preprocess_weights(inp):
    """Fold the plate linear algebra host-side (float64 -> bf16/f32)."""
    f64 = np.float64
    W_tr = np.asarray(inp["W_tr"], f64)
    b_tr = np.asarray(inp["b_tr"], f64)
    W_ab = np.asarray(inp["W_ab"], f64)
    b_ab = np.asarray(inp["b_ab"], f64)
    W_eq = np.asarray(inp["W_eq"], f64)
    b_eq = np.asarray(inp["b_eq"], f64)
    W1 = np.asarray(inp["W1"], f64)
    b1 = np.asarray(inp["b1"], f64)
    b2 = np.asarray(inp["b2"], f64)
    alpha = float(np.asarray(inp["alpha"]))

    W_trp = alpha * W_tr
    ab_tr = alpha * b_tr
    W_trab = W_trp @ W_ab
    c2 = ab_tr @ W_ab + b_ab
    W_desc = W_trab @ W_eq
    c3 = c2 @ W_eq
    W1_g, W1_l = W1[:D_GAS], W1[D_GAS:]
    W_fold = W_trab @ W1_l
    A = -W_trp
    c = -ab_tr

    # gamma-fold: gamma_n = gamma_{n-1} + gamma_{n-1} @ A + c, gamma_0 = 0
    gammas = [np.zeros(D_GAS)]
    for _n in range(1, N_PLATES + 1):
        g_ = gammas[-1]
        gammas.append(g_ + g_ @ A + c)

    # sigmoid biases per (sweep, plate): z_n = pacc_n + be[s, n]
    be0 = np.stack([b_eq + (9 - n) * c3 for n in range(1, 9)])
    be1 = np.stack(
        [
            b_eq
            + (9 - n) * c3
            + sum(gammas[m - 1] for m in range(n, 9)) @ W_desc
            for n in range(1, 9)
        ]
    )
    be = np.stack([be0, be1])  # (2, 8, 256)
    # layout [128, 32]: col = s*16 + (n-1)*2 + ft; feature f = ft*128 + p
    be_cols = np.empty((128, 32), np.float32)
    for s in range(2):
        for n in range(8):
            for ft in range(2):
                be_cols[:, s * 16 + n * 2 + ft] = be[s, n, ft * 128 : (ft + 1) * 128]

    e9 = 1.0 / (1.0 + np.exp(-b_eq))
    hb = (
        b1
        + 8.0 * (c2 @ W1_l)
        + gammas[8] @ W1_g
        + sum(gammas[m - 1] for m in range(1, 9)) @ W_fold
    )

    def per_ft(v, dtype=np.float32):
        return np.ascontiguousarray(np.asarray(v, np.float64).reshape(2, 128).T.astype(dtype))

    def w_bf(w):
        return np.ascontiguousarray(np.asarray(w, NPBF16))

    return {
        "wge": w_bf(inp["W_ge"]),
        "wdesc": w_bf(W_desc),
        "wasc": w_bf(A),
        "wfold": w_bf(W_fold),
        "w1g": w_bf(W1_g),
        "w2": w_bf(inp["W2"]),
        "b2r": w_bf(b2.reshape(1, D_OUT)),
        "be": np.ascontiguousarray(be_cols),
        "ne9": per_ft(-e9),
        "bge": per_ft(inp["b_ge"]),
        "hb": per_ft(hb),
        "iden": np.eye(128, dtype=NPBF16),
        "ones": np.ones((1, 128), NPBF16),
    }


def build_nc():
    nc = bacc.Bacc("TRN2", target_bir_lowering=False, debug=False)

    x_d = nc.dram_tensor("x", (ROWS, D_IN), BF16, kind="ExternalInput").ap()
    wge_d = nc.dram_tensor("wge", (D_IN, D_GAS), BF16, kind="ExternalInput").ap()
    wdesc_d = nc.dram_tensor("wdesc", (D_GAS, D_GAS), BF16, kind="ExternalInput").ap()
    wasc_d = nc.dram_tensor("wasc", (D_GAS, D_GAS), BF16, kind="ExternalInput").ap()
    wfold_d = nc.dram_tensor("wfold", (D_GAS, D_GAS), BF16, kind="ExternalInput").ap()
    w1g_d = nc.dram_tensor("w1g", (D_GAS, D_GAS), BF16, kind="ExternalInput").ap()
    w2_d = nc.dram_tensor("w2", (D_GAS, D_OUT), BF16, kind="ExternalInput").ap()
    b2r_d = nc.dram_tensor("b2r", (1, D_OUT), BF16, kind="ExternalInput").ap()
    be_d = nc.dram_tensor("be", (128, 32), F32, kind="ExternalInput").ap()
    ne9_d = nc.dram_tensor("ne9", (128, 2), F32, kind="ExternalInput").ap()
    bge_d = nc.dram_tensor("bge", (128, 2), F32, kind="ExternalInput").ap()
    hb_d = nc.dram_tensor("hb", (128, 2), F32, kind="ExternalInput").ap()
    iden_d = nc.dram_tensor("iden", (128, 128), BF16, kind="ExternalInput").ap()
    ones_d = nc.dram_tensor("ones", (1, 128), BF16, kind="ExternalInput").ap()
    out_d = nc.dram_tensor("out", (ROWS, D_OUT), F32, kind="ExternalOutput").ap()

    with tile.TileContext(nc) as tc:
        with (
            tc.tile_pool(name="const", bufs=1) as cpool,
            tc.tile_pool(name="state", bufs=1) as spool,
            tc.tile_pool(name="work", bufs=3) as wpool,
            tc.tile_pool(name="psum", bufs=1, space="PSUM") as ppool,
        ):
            # ---- x first (XBAR transposes gate the encoder), then weights ----
            xT = []
            for c in range(N_CHUNKS):
                r0 = c * R
                xt = wpool.tile([128, 4, R], BF16, tag=f"xT{c}", bufs=1, name=f"xT{c}")
                for k in range(4):
                    nc.sync.dma_start_transpose(
                        xt[:, k, :], x_d[r0 : r0 + R, k * 128 : (k + 1) * 128]
                    )
                xT.append(xt)

            wge_t = cpool.tile([128, 4, D_GAS], BF16, tag="wge")
            nc.sync.dma_start(wge_t, wge_d.rearrange("(ko ki) m -> ki ko m", ki=128))
            bge_t = cpool.tile([128, 2], F32, tag="bge")
            nc.sync.dma_start(bge_t, bge_d)
            wdesc_t = cpool.tile([128, 2, D_GAS], BF16, tag="wdesc")
            nc.sync.dma_start(wdesc_t, wdesc_d.rearrange("(ko ki) m -> ki ko m", ki=128))
            be_t = cpool.tile([128, 32], F32, tag="be")
            nc.sync.dma_start(be_t, be_d)
            ne9_t = cpool.tile([128, 2], F32, tag="ne9")
            nc.sync.dma_start(ne9_t, ne9_d)
            wasc_t = cpool.tile([128, 2, D_GAS], BF16, tag="wasc")
            nc.sync.dma_start(wasc_t, wasc_d.rearrange("(ko ki) m -> ki ko m", ki=128))
            iden_t = cpool.tile([128, 128], BF16, tag="iden")
            nc.sync.dma_start(iden_t, iden_d)
            wfold_t = cpool.tile([128, 2, D_GAS], BF16, tag="wfold")
            nc.sync.dma_start(wfold_t, wfold_d.rearrange("(ko ki) m -> ki ko m", ki=128))
            w1g_t = cpool.tile([128, 2, D_GAS], BF16, tag="w1g")
            nc.sync.dma_start(w1g_t, w1g_d.rearrange("(ko ki) m -> ki ko m", ki=128))
            w2_t = cpool.tile([128, 2, D_OUT], BF16, tag="w2")
            nc.sync.dma_start(w2_t, w2_d.rearrange("(ko ki) n -> ki ko n", ki=128))
            b2r_t = cpool.tile([1, D_OUT], BF16, tag="b2r")
            nc.sync.dma_start(b2r_t, b2r_d)
            hb_t = cpool.tile([128, 2], F32, tag="hb")
            nc.sync.dma_start(hb_t, hb_d)
            ones_t = cpool.tile([1, 128], BF16, tag="ones")
            nc.sync.dma_start(ones_t, ones_d)

            def be_ap(s, n, ft):
                col = s * 16 + (n - 1) * 2 + ft
                return be_t[:, col : col + 1]

            # ---- per-chunk state tiles ----
            st_g = [[None] * (N_PLATES + 1) for _ in range(N_CHUNKS)]
            st_e = [[None] * (N_PLATES + 2) for _ in range(N_CHUNKS)]
            S = [None] * N_CHUNKS
            h = [None] * N_CHUNKS
            for c in range(N_CHUNKS):
                for n in range(N_PLATES + 1):
                    st_g[c][n] = spool.tile([128, 2, R], BF16, tag=f"g{c}_{n}", name=f"g{c}_{n}")
                for n in range(1, N_PLATES + 1):
                    st_e[c][n] = spool.tile([128, 2, R], BF16, tag=f"e{c}_{n}", name=f"e{c}_{n}")
                S[c] = spool.tile([128, 2, R], BF16, tag=f"S{c}", name=f"S{c}")
                h[c] = spool.tile([128, 2, R], BF16, tag=f"h{c}", name=f"h{c}")

            # ---- encoder ----
            for c in range(N_CHUNKS):
                acc = ppool.tile([128, 2, R], F32, tag=f"acc{c}")
                for ft in range(2):
                    for n0 in (0, 512):
                        for k in range(4):
                            nc.tensor.matmul(
                                acc[:, ft, n0 : n0 + 512],
                                lhsT=wge_t[:, k, ft * 128 : (ft + 1) * 128],
                                rhs=xT[c][:, k, n0 : n0 + 512],
                                start=(k == 0),
                                stop=(k == 3),
                            )
                for ft in range(2):
                    nc.scalar.activation(
                        st_g[c][0][:, ft, :], acc[:, ft, :], AF.Relu,
                        bias=bge_t[:, ft : ft + 1],
                    )

            # ---- sweeps ----
            for s in range(2):
                # descending
                pacc = [ppool.tile([128, 2, R], F32, tag=f"acc{c}", name=f"pacc{c}_{s}")
                        for c in range(N_CHUNKS)]
                for n in range(N_PLATES, 0, -1):
                    for c in range(N_CHUNKS):
                        df = wpool.tile([128, 2, R], BF16, tag="df", bufs=4)
                        if n == N_PLATES:
                            gp = st_g[c][0] if s == 0 else st_g[c][7]
                            for ft in range(2):
                                nc.vector.tensor_scalar(
                                    df[:, ft, :], gp[:, ft, :],
                                    ne9_t[:, ft : ft + 1], None, OP.add,
                                )
                        else:
                            gp = st_g[c][0] if s == 0 else st_g[c][n - 1]
                            nc.vector.tensor_tensor(
                                df, gp, st_e[c][n + 1], OP.subtract
                            )
                        if s == 1:
                            if n == N_PLATES:
                                nc.vector.tensor_copy(S[c], df)
                            else:
                                nc.vector.tensor_tensor(S[c], S[c], df, OP.add)
                        for ft in range(2):
                            for n0 in (0, 512):
                                for k in range(2):
                                    nc.tensor.matmul(
                                        pacc[c][:, ft, n0 : n0 + 512],
                                        lhsT=wdesc_t[:, k, ft * 128 : (ft + 1) * 128],
                                        rhs=df[:, k, n0 : n0 + 512],
                                        start=(n == N_PLATES and k == 0),
                                        stop=(n == 1 and k == 1),
                                        skip_group_check=True,
                                    )
                        for ft in range(2):
                            nc.scalar.activation(
                                st_e[c][n][:, ft, :], pacc[c][:, ft, :],
                                AF.Sigmoid, bias=be_ap(s, n, ft),
                            )

                # ascending: gas state accumulates in PSUM, seeded with g0
                gps = []
                for c in range(N_CHUNKS):
                    g_psum = ppool.tile([128, 2, R], F32, tag=f"acc{c}")
                    for ft in range(2):
                        for n0 in (0, 512):
                            nc.tensor.matmul(
                                g_psum[:, ft, n0 : n0 + 512],
                                lhsT=iden_t,
                                rhs=st_g[c][0][:, ft, n0 : n0 + 512],
                                start=True,
                                stop=False,
                                skip_group_check=True,
                            )
                    gps.append(g_psum)
                for n in range(1, N_PLATES + 1):
                    for c in range(N_CHUNKS):
                        df = wpool.tile([128, 2, R], BF16, tag="df", bufs=4)
                        nc.vector.tensor_tensor(
                            df, st_g[c][n - 1], st_e[c][n], OP.subtract
                        )
                        for ft in range(2):
                            for n0 in (0, 512):
                                for k in range(2):
                                    nc.tensor.matmul(
                                        gps[c][:, ft, n0 : n0 + 512],
                                        lhsT=wasc_t[:, k, ft * 128 : (ft + 1) * 128],
                                        rhs=df[:, k, n0 : n0 + 512],
                                        start=False,
                                        stop=(n == N_PLATES and k == 1),
                                        skip_group_check=True,
                                    )
                        nc.scalar.activation(
                            st_g[c][n], gps[c], AF.Copy,
                        )

            # ---- head ----
            for c in range(N_CHUNKS):
                ph = ppool.tile([128, 2, R], F32, tag=f"acc{c}")
                for ft in range(2):
                    for n0 in (0, 512):
                        for k in range(2):
                            nc.tensor.matmul(
                                ph[:, ft, n0 : n0 + 512],
                                lhsT=w1g_t[:, k, ft * 128 : (ft + 1) * 128],
                                rhs=st_g[c][N_PLATES][:, k, n0 : n0 + 512],
                                start=(k == 0),
                                stop=False,
                            )
                        for k in range(2):
                            nc.tensor.matmul(
                                ph[:, ft, n0 : n0 + 512],
                                lhsT=wfold_t[:, k, ft * 128 : (ft + 1) * 128],
                                rhs=S[c][:, k, n0 : n0 + 512],
                                start=False,
                                stop=(k == 1),
                            )
                for ft in range(2):
                    nc.scalar.activation(
                        h[c][:, ft, :], ph[:, ft, :], AF.Relu,
                        bias=hb_t[:, ft : ft + 1],
                    )

            # ---- output head matmul: out = h @ W2 + b2 ----
            # rb-pairs interleaved across chunks so one chunk's matmuls run
            # while the other's stage copies / DMAs drain.
            for rbp in range(R // 256):
                for c in range(N_CHUNKS):
                    r0 = c * R
                    po = ppool.tile([128, 2, R], F32, tag=f"acc{c}",
                                    name=f"po{c}_{rbp}")
                    for par in range(2):
                        rb = rbp * 2 + par
                        for n0, nw in ((0, 512), (512, 488)):
                            for ft in range(2):
                                nc.tensor.matmul(
                                    po[:, par, n0 : n0 + nw],
                                    lhsT=h[c][:, ft, rb * 128 : (rb + 1) * 128],
                                    rhs=w2_t[:, ft, n0 : n0 + nw],
                                    start=(ft == 0),
                                    stop=False,
                                )
                            nc.tensor.matmul(
                                po[:, par, n0 : n0 + nw],
                                lhsT=ones_t[0:1, 0:128],
                                rhs=b2r_t[0:1, n0 : n0 + nw],
                                start=False,
                                stop=True,
                            )
                    for par in range(2):
                        rb = rbp * 2 + par
                        stage = wpool.tile([128, D_OUT], F32, tag="stage", bufs=2)
                        if par == 0:
                            nc.vector.tensor_copy(stage, po[:, par, 0:D_OUT])
                        else:
                            nc.scalar.activation(stage, po[:, par, 0:D_OUT], AF.Copy)
                        nc.sync.dma_start(
                            out_d[r0 + rb * 128 : r0 + (rb + 1) * 128, :], stage
                        )

    nc.compile()
    return nc


_NC_CACHE = {}


def kernel(**inputs):
    inp = {k: np.asarray(v) for k, v in inputs.items()}
    prep = _preprocess_weights(inp)
    x = np.ascontiguousarray(np.asarray(inp["x"], np.float32).astype(NPBF16))

    if "nc" not in _NC_CACHE:
        _NC_CACHE["nc"] = build_nc()
    nc = _NC_CACHE["nc"]

    in_maps = []
    for c in range(N_CORES):
        m = {"x": x[c * ROWS : (c + 1) * ROWS]}
        m.update(prep)
        in_maps.append(m)
    res = bass_utils.run_bass_kernel_spmd(nc, in_maps, core_ids=list(range(N_CORES)))
    out = np.concatenate([res.results[c]["out"] for c in range(N_CORES)], axis=0)
    return out
